# revision 1
# baseline (speedup 1.0000x reference)
"""Trainium2 Bass kernel for DeformableTokenEmbedding.

Full shapes: x [32, 36864, 16] f32, w_off [48,16,24], b_off [48],
w_def [512,16,24], b_def [512] -> out [32, 1536, 512] f32.

Strategy: pure data parallel over batch (4 batches per core x 8 cores).

Math (per batch):
  View x as V0 [Lout, M] with M = K*C = 384, m = k*C + c (flat memory view).
  offT[o, l] = sum_m wo2[m, o] * V0T[m, l]       (offset conv, PE)
  dy = offT rows 0..23, dx = rows 24..47 (host-permuted weight columns)
  wy  = relu(1 - |dy|) = relu(min(1-dy, 1+dy))
  u_p = wy*relu(dx); u_m = wy*relu(-dx); u_0 = wy - u_p - u_m
  V2w[l, m] = u_0[l,k]*x0 + u_m[l,k]*xm + u_p[l,k]*xp   (3-tap bilinear, DVE)
    where x0/xm/xp are +-C shifted flat views of x (exact zero-padded
    bilinear for |dx| <= 1; dx std is ~0.2 so this is exact in practice)
  out[l, d] = sum_m V2wT[m, l] * wd2[m, d] + b_def[d]   (output GEMM, PE)

b_def is added on the host (free) after gathering shards.
"""

from contextlib import ExitStack

import numpy as np
import ml_dtypes

import concourse.bass as bass
import concourse.tile as tile
from concourse import mybir, bacc
from concourse.bass_utils import run_bass_kernel_spmd

# problem constants
B, L, C, D, K = 32, 36864, 16, 512, 24
LOUT = L // K          # 1536
M = K * C              # 384
NCORES = 8
BPC = B // NCORES      # 4 batches per core

F32 = mybir.dt.float32
BF16 = mybir.dt.bfloat16
TT = mybir.AluOpType


DEFAULT_CFG = dict(lp=4, wp=5, vp=6, xb=4, xt=8, vts=8, tr=2, op=1)


def build_kernel(bpc=BPC, lout=LOUT, lchunk=512, d=D, dbg=False, cfg=None):
    cfg = dict(DEFAULT_CFG, **(cfg or {}))
    g = cfg.get
    """Build the per-core Bass program. lchunk must divide lout and be a
    multiple of 128."""
    nct = lchunk // 128            # l-tiles per chunk
    nlc = lout // lchunk           # chunks per batch
    nmc = M // 128                 # m-chunks (3)

    nc = bacc.Bacc("TRN2", target_bir_lowering=False, debug=False,
                   num_devices=NCORES)

    x_in = nc.dram_tensor("x", [bpc, lout + 2, M], BF16, kind="ExternalInput")
    wo2_in = nc.dram_tensor("wo2", [M, 64], BF16, kind="ExternalInput")
    bo2_in = nc.dram_tensor("bo2", [48, 1], F32, kind="ExternalInput")
    wd2_in = nc.dram_tensor("wd2", [M, d], BF16, kind="ExternalInput")
    idn_in = nc.dram_tensor("idn", [128, 128], BF16, kind="ExternalInput")
    out_dram = nc.dram_tensor("out", [bpc, lout, d], F32, kind="ExternalOutput")
    if dbg:
        nct0 = lchunk // 128
        dbg_xt = nc.dram_tensor("dbg_xt", [128, lchunk], BF16, kind="ExternalOutput")
        dbg_dy = nc.dram_tensor("dbg_dy", [24, lchunk], BF16, kind="ExternalOutput")
        dbg_dx = nc.dram_tensor("dbg_dx", [24, lchunk], BF16, kind="ExternalOutput")
        dbg_u3n = nc.dram_tensor("dbg_u3n", [128, 144 * nct0], BF16, kind="ExternalOutput")
        dbg_v2w = nc.dram_tensor("dbg_v2w", [128, M], BF16, kind="ExternalOutput")
        dbg_vts = nc.dram_tensor("dbg_vts", [128, lchunk], BF16, kind="ExternalOutput")

    x_nat = x_in.ap()  # [bpc, lout+2, M], zero row at each end (host-padded)

    with tile.TileContext(nc) as tc, ExitStack() as ctx:
        cpool = ctx.enter_context(tc.tile_pool(name="consts", bufs=1))
        lpool = ctx.enter_context(tc.tile_pool(name="loads", bufs=g("lp", 3)))
        wpool = ctx.enter_context(tc.tile_pool(name="work", bufs=g("wp", 2)))
        vpool = ctx.enter_context(tc.tile_pool(name="vals", bufs=g("vp", 3)))
        trpool = ctx.enter_context(tc.tile_pool(name="ptr", bufs=g("tr", 1), space="PSUM"))
        offpool = ctx.enter_context(tc.tile_pool(name="poff", bufs=1, space="PSUM"))
        upool = ctx.enter_context(tc.tile_pool(name="pu", bufs=1, space="PSUM"))
        t2pool = ctx.enter_context(tc.tile_pool(name="pt2", bufs=g("t2", 3), space="PSUM"))
        opool = ctx.enter_context(tc.tile_pool(name="pout", bufs=g("op", 2), space="PSUM"))

        # constants
        wo2 = []
        wd2 = []
        for mc in range(nmc):
            t_wo2 = cpool.tile([128, 64], BF16, tag=f"wo2{mc}", name=f"wo2_{mc}")
            wo2.append(t_wo2)
            t_wd2 = cpool.tile([128, d], BF16, tag=f"wd2{mc}", name=f"wd2_{mc}")
            wd2.append(t_wd2)
        for mc in range(nmc):
            nc.sync.dma_start(wo2[mc][:], wo2_in[mc * 128:(mc + 1) * 128, :])
            nc.sync.dma_start(wd2[mc][:], wd2_in[mc * 128:(mc + 1) * 128, :])
        boy = cpool.tile([24, 1], F32, tag="boy")
        nc.sync.dma_start(boy[:], bo2_in[0:24, :])
        box32 = cpool.tile([56, 1], F32, tag="box32")
        nc.sync.dma_start(box32[32:56], bo2_in[24:48, :])
        ident = cpool.tile([128, 128], BF16, tag="ident")
        nc.sync.dma_start(ident[:], idn_in[:])


        for b in range(bpc):
            for lc in range(nlc):
                l0 = lc * lchunk
                # ---- load: one DMA per chunk (halo'd windows) ----
                W = 16 + M + 16
                xbw = lpool.tile([128, nct * W], BF16, tag="xbw",
                                 bufs=g("xb", 3))
                # one DMA for the whole chunk: overlapping 416-wide windows
                src = x_nat[b, 1 + l0:1 + l0 + 128, :]
                src = src[:, None, :].broadcast_to((128, nct, M))
                # [p, i, w]: p step=M rows, i step=128*M rows, w step 1,
                # offset shifted -16 for the left halo
                src.ap = mybir.VecI64Pair(
                    [[M, 128], [128 * M, nct], [1, W]])
                src.offset = src.offset - 16
                dst = xbw[:].rearrange("p (i w) -> p i w", i=nct)
                nc.sync.dma_start(dst, src)
                xbs = [xbw[:, i * W:i * W + W] for i in range(nct)]

                # ---- T1: transpose raw x to [m, l] for the offset conv ----
                xt = []
                for mc in range(nmc):
                    tr = trpool.tile([128, lchunk], BF16, tag="tr")
                    for i in range(nct):
                        nc.tensor.transpose(
                            tr[:, i * 128:(i + 1) * 128],
                            xbs[i][:, 16 + mc * 128:16 + (mc + 1) * 128],
                            ident[:])
                    xts = wpool.tile([128, lchunk], BF16, tag="xt", bufs=g("xt", 4))
                    nc.scalar.copy(xts[:], tr[:])
                    xt.append(xts)
                if dbg and b == 0 and lc == 0:
                    nc.sync.dma_start(dbg_xt[:], xt[0][:])

                # ---- stage A: offset conv ----
                offps = offpool.tile([64, lchunk], F32, tag="offps")
                for mc in range(nmc):
                    nc.tensor.matmul(offps[:], wo2[mc][:], xt[mc][:],
                                     start=(mc == 0), stop=(mc == nmc - 1))
                dyt = wpool.tile([24, lchunk], BF16, tag="dyt")
                nc.scalar.activation(dyt[:], offps[0:24],
                                     mybir.ActivationFunctionType.Identity,
                                     bias=boy[:], scale=1.0)
                if dbg and b == 0 and lc == 0:
                    nc.sync.dma_start(dbg_dy[:], dyt[:])

                # ---- u pipeline, all on partitions 32:56 ----
                dy32 = wpool.tile([56, lchunk], BF16, tag="dy32")
                nc.sync.dma_start(dy32[32:56], dyt[:])
                dy = dy32[32:56]
                s1 = wpool.tile([56, lchunk], BF16, tag="s1", name="s1")[32:56]
                nc.vector.tensor_scalar(s1, dy, -1.0, 1.0, TT.mult, TT.add)
                s2 = wpool.tile([56, lchunk], BF16, tag="s2", name="s2")[32:56]
                nc.vector.tensor_scalar_add(s2, dy, 1.0)
                wy = wpool.tile([56, lchunk], BF16, tag="wy", name="wy")[32:56]
                nc.vector.tensor_tensor(out=wy, in0=s1, in1=s2, op=TT.min)
                nc.vector.tensor_scalar_max(wy, wy, 0.0)
                dxt = wpool.tile([56, lchunk], BF16, tag="dxt", name="dxt")[32:56]
                nc.vector.tensor_scalar(dxt, offps[32:56], 1.0, box32[32:56],
                                        TT.mult, TT.add)
                rp = wpool.tile([56, lchunk], BF16, tag="rp", name="rp")[32:56]
                nc.vector.tensor_scalar_max(rp, dxt, 0.0)
                rm = wpool.tile([56, lchunk], BF16, tag="rm", name="rm")[32:56]
                nc.vector.tensor_scalar(rm, dxt, -1.0, 0.0, TT.mult, TT.max)
                u_m = wpool.tile([56, lchunk], BF16, tag="u_m", name="u_m")[32:56]
                u_0 = wpool.tile([56, lchunk], BF16, tag="u_0", name="u_0")[32:56]
                u_p = wpool.tile([56, lchunk], BF16, tag="u_p", name="u_p")[32:56]
                nc.vector.tensor_tensor(out=u_p, in0=wy, in1=rp, op=TT.mult)
                nc.vector.tensor_tensor(out=u_m, in0=wy, in1=rm, op=TT.mult)
                t0 = wpool.tile([56, lchunk], BF16, tag="t0", name="t0")[32:56]
                nc.vector.tensor_tensor(out=t0, in0=wy, in1=u_p, op=TT.subtract)
                nc.vector.tensor_tensor(out=u_0, in0=t0, in1=u_m, op=TT.subtract)
                u_tiles = [u_m, u_0, u_p]

                # ---- transpose u3 to natural [l, (j k)] ----
                upsum = upool.tile([128, 72 * nct], BF16, tag="upsum")
                for i in range(nct):
                    for jj in range(3):
                        nc.tensor.transpose(
                            upsum[:, i * 72 + jj * 24:i * 72 + jj * 24 + 24],
                            u_tiles[jj][:, i * 128:(i + 1) * 128],
                            ident[32:56, 32:56])
                u3n = wpool.tile([128, 144 * nct], BF16, tag="u3n")
                usrc = upsum[:].rearrange("p (i r) -> p i r", i=nct)
                usrc = usrc[:, :, :, None].broadcast_to((128, nct, 72, 2))
                nc.scalar.copy(
                    u3n[:].rearrange("p (i r t) -> p i r t", i=nct, r=72),
                    usrc)
                if dbg and b == 0 and lc == 0:
                    nc.sync.dma_start(dbg_u3n[:], u3n[:])

                # ---- weighting (natural domain) + T2 per l-tile ----
                vt = []
                for mc in range(nmc):
                    t_vt = t2pool.tile([128, lchunk], BF16, tag="tr2", name=f"vt_{mc}")
                    vt.append(t_vt)
                for i in range(nct):
                    xb = xbs[i]
                    ub = u3n[:, i * 144:(i + 1) * 144]
                    pbuf = vpool.tile([128, 3 * M], BF16, tag="pbuf")
                    for jj in range(3):
                        # U pairs: u3 rows 32*jj.. -> cols 64*jj..64*jj+48,
                        # each value duplicated (2k, 2k+1)
                        uv = ub[:, 48 * jj:48 * jj + 48].rearrange(
                            "p (k t) -> p k t", k=K)
                        uv = uv[:, :, None, :].broadcast_to((128, K, 8, 2))
                        xv = xb[:, 16 * jj:16 * jj + M].rearrange(
                            "p (k c8 c2) -> p k c8 c2", k=K, c8=8)
                        pv = pbuf[:, jj * M:(jj + 1) * M].rearrange(
                            "p (k c8 c2) -> p k c8 c2", k=K, c8=8)
                        nc.vector.tensor_tensor(out=pv, in0=xv, in1=uv,
                                                op=TT.mult)
                    v2w = vpool.tile([128, M], BF16, tag="v2w")
                    add_eng = nc.gpsimd if g("gadd", 0) else nc.vector
                    add_eng.tensor_tensor(out=v2w[:], in0=pbuf[:, 0:M],
                                          in1=pbuf[:, M:2 * M], op=TT.add)
                    add_eng.tensor_tensor(out=v2w[:], in0=v2w[:],
                                          in1=pbuf[:, 2 * M:3 * M], op=TT.add)
                    if dbg and b == 0 and lc == 0 and i == 0:
                        nc.sync.dma_start(dbg_v2w[:], v2w[:])
                    for mc in range(nmc):
                        nc.tensor.transpose(
                            vt[mc][:, i * 128:(i + 1) * 128],
                            v2w[:, mc * 128:(mc + 1) * 128],
                            ident[:])
                vts = []
                for mc in range(nmc):
                    v = vpool.tile([128, lchunk], BF16, tag="vts", bufs=g("vts", 4))
                    nc.scalar.copy(v[:], vt[mc][:])
                    vts.append(v)
                if dbg and b == 0 and lc == 0:
                    nc.sync.dma_start(dbg_vts[:], vts[0][:])
                # ---- stage C ----
                osb4 = vpool.tile([128, nct * d], F32, tag="osb4",
                                  bufs=g("osb", 2))
                for i in range(nct):
                    outps = opool.tile([128, d], F32, tag="outps")
                    for mc in range(nmc):
                        nc.tensor.matmul(outps[:],
                                         vts[mc][:, i * 128:(i + 1) * 128],
                                         wd2[mc][:],
                                         start=(mc == 0),
                                         stop=(mc == nmc - 1))
                    nc.scalar.copy(osb4[:, i * d:(i + 1) * d], outps[:])
                odst = out_dram[b, l0:l0 + lchunk, :].rearrange(
                    "(i p) d -> p i d", p=128)
                nc.sync.dma_start(odst, osb4[:].rearrange(
                    "p (i d) -> p i d", i=nct))

    nc.compile()
    return nc


def prep_weights(w_off, b_off, w_def):
    """Host-side weight rearrangement. wo2[k*C+c, o'] with o' 0..23 = dy_k
    (w_off channel 2k), o' 24..47 = dx_k (channel 2k+1)."""
    d = w_def.shape[0]
    wo2 = np.zeros((M, 64), np.float32)
    wd2 = np.zeros((M, d), np.float32)
    bo2 = np.zeros((48, 1), np.float32)
    for k in range(K):
        for c in range(C):
            m = k * C + c
            wo2[m, 0:24] = w_off[0::2, c, k]
            wo2[m, 32:56] = w_off[1::2, c, k]
            wd2[m, :] = w_def[:, c, k]
    bo2[0:24, 0] = b_off[0::2]
    bo2[24:48, 0] = b_off[1::2]
    return (wo2.astype(ml_dtypes.bfloat16), bo2,
            wd2.astype(ml_dtypes.bfloat16))


def make_identity():
    return np.eye(128, dtype=ml_dtypes.bfloat16)


def make_identity2():
    i2 = np.zeros((96, 192), ml_dtypes.bfloat16)
    for r in range(96):
        i2[r, 2 * r] = 1
        i2[r, 2 * r + 1] = 1
    return i2


_NC_CACHE = {}


def pad_x(x_shard):
    bpc = x_shard.shape[0]
    lout = x_shard.shape[1] // K
    xp = np.zeros((bpc, lout + 2, M), ml_dtypes.bfloat16)
    xp[:, 1:-1, :] = x_shard.reshape(bpc, lout, M).astype(ml_dtypes.bfloat16)
    return xp


def kernel(x, w_off, b_off, w_def, b_def, trace=False):
    x = np.ascontiguousarray(np.asarray(x, np.float32))
    wo2, bo2, wd2 = prep_weights(np.asarray(w_off, np.float32),
                                 np.asarray(b_off, np.float32),
                                 np.asarray(w_def, np.float32))
    idn = make_identity()
    if "nc" not in _NC_CACHE:
        _NC_CACHE["nc"] = build_kernel()
    nc = _NC_CACHE["nc"]
    in_maps = []
    for r in range(NCORES):
        in_maps.append({
            "x": pad_x(x[r * BPC:(r + 1) * BPC]),
            "wo2": wo2, "bo2": bo2, "wd2": wd2, "idn": idn,
        })
    try:
        res = run_bass_kernel_spmd(nc, in_maps, core_ids=list(range(NCORES)),
                                   trace=trace)
    except (ImportError, ModuleNotFoundError):
        res = run_bass_kernel_spmd(nc, in_maps, core_ids=list(range(NCORES)))
    out = np.concatenate([res.results[r]["out"] for r in range(NCORES)], axis=0)
    out = out + np.asarray(b_def, np.float32)[None, None, :]
    if trace:
        return out.astype(np.float32), res
    return out.astype(np.float32)



# revision 4
# speedup vs baseline: 1.0109x; 1.0109x over previous
"""Trainium2 Bass kernel for DeformableTokenEmbedding.

Full shapes: x [32, 36864, 16] f32, w_off [48,16,24], b_off [48],
w_def [512,16,24], b_def [512] -> out [32, 1536, 512] f32.

Strategy: pure data parallel over batch (4 batches per core x 8 cores).

Math (per batch):
  View x as V0 [Lout, M] with M = K*C = 384, m = k*C + c (flat memory view).
  offT[o, l] = sum_m wo2[m, o] * V0T[m, l]       (offset conv, PE)
  dy = offT rows 0..23, dx = rows 24..47 (host-permuted weight columns)
  wy  = relu(1 - |dy|) = relu(min(1-dy, 1+dy))
  u_p = wy*relu(dx); u_m = wy*relu(-dx); u_0 = wy - u_p - u_m
  V2w[l, m] = u_0[l,k]*x0 + u_m[l,k]*xm + u_p[l,k]*xp   (3-tap bilinear, DVE)
    where x0/xm/xp are +-C shifted flat views of x (exact zero-padded
    bilinear for |dx| <= 1; dx std is ~0.2 so this is exact in practice)
  out[l, d] = sum_m V2wT[m, l] * wd2[m, d] + b_def[d]   (output GEMM, PE)

b_def is added on the host (free) after gathering shards.
"""

from contextlib import ExitStack

import numpy as np
import ml_dtypes

import concourse.bass as bass
import concourse.tile as tile
from concourse import mybir, bacc
from concourse.bass_utils import run_bass_kernel_spmd

# problem constants
B, L, C, D, K = 32, 36864, 16, 512, 24
LOUT = L // K          # 1536
M = K * C              # 384
NCORES = 8
BPC = B // NCORES      # 4 batches per core

F32 = mybir.dt.float32
BF16 = mybir.dt.bfloat16
TT = mybir.AluOpType


DEFAULT_CFG = dict(lp=4, wp=5, vp=6, xb=4, xt=8, vts=8, tr=2, op=1)


def build_kernel(bpc=BPC, lout=LOUT, lchunk=512, d=D, dbg=False, cfg=None):
    cfg = dict(DEFAULT_CFG, **(cfg or {}))
    g = cfg.get
    """Build the per-core Bass program. lchunk must divide lout and be a
    multiple of 128."""
    nct = lchunk // 128            # l-tiles per chunk
    nlc = lout // lchunk           # chunks per batch
    nmc = M // 128                 # m-chunks (3)

    nc = bacc.Bacc("TRN2", target_bir_lowering=False, debug=False,
                   num_devices=NCORES)

    x_in = nc.dram_tensor("x", [bpc, lout + 2, M], BF16, kind="ExternalInput")
    wo2_in = nc.dram_tensor("wo2", [M, 64], BF16, kind="ExternalInput")
    bo2_in = nc.dram_tensor("bo2", [48, 1], F32, kind="ExternalInput")
    wd2_in = nc.dram_tensor("wd2", [M, d], BF16, kind="ExternalInput")
    idn_in = nc.dram_tensor("idn", [128, 128], BF16, kind="ExternalInput")
    out_dram = nc.dram_tensor("out", [bpc, lout, d], BF16, kind="ExternalOutput")
    if dbg:
        nct0 = lchunk // 128
        dbg_xt = nc.dram_tensor("dbg_xt", [128, lchunk], BF16, kind="ExternalOutput")
        dbg_dy = nc.dram_tensor("dbg_dy", [24, lchunk], BF16, kind="ExternalOutput")
        dbg_dx = nc.dram_tensor("dbg_dx", [24, lchunk], BF16, kind="ExternalOutput")
        dbg_u3n = nc.dram_tensor("dbg_u3n", [128, 144 * nct0], BF16, kind="ExternalOutput")
        dbg_v2w = nc.dram_tensor("dbg_v2w", [128, M], BF16, kind="ExternalOutput")
        dbg_vts = nc.dram_tensor("dbg_vts", [128, lchunk], BF16, kind="ExternalOutput")

    x_nat = x_in.ap()  # [bpc, lout+2, M], zero row at each end (host-padded)

    with tile.TileContext(nc) as tc, ExitStack() as ctx:
        cpool = ctx.enter_context(tc.tile_pool(name="consts", bufs=1))
        lpool = ctx.enter_context(tc.tile_pool(name="loads", bufs=g("lp", 3)))
        wpool = ctx.enter_context(tc.tile_pool(name="work", bufs=g("wp", 2)))
        vpool = ctx.enter_context(tc.tile_pool(name="vals", bufs=g("vp", 3)))
        trpool = ctx.enter_context(tc.tile_pool(name="ptr", bufs=g("tr", 1), space="PSUM"))
        offpool = ctx.enter_context(tc.tile_pool(name="poff", bufs=1, space="PSUM"))
        upool = ctx.enter_context(tc.tile_pool(name="pu", bufs=1, space="PSUM"))
        t2pool = ctx.enter_context(tc.tile_pool(name="pt2", bufs=g("t2", 3), space="PSUM"))
        opool = ctx.enter_context(tc.tile_pool(name="pout", bufs=g("op", 2), space="PSUM"))

        # constants
        wo2 = []
        wd2 = []
        for mc in range(nmc):
            t_wo2 = cpool.tile([128, 64], BF16, tag=f"wo2{mc}", name=f"wo2_{mc}")
            wo2.append(t_wo2)
            t_wd2 = cpool.tile([128, d], BF16, tag=f"wd2{mc}", name=f"wd2_{mc}")
            wd2.append(t_wd2)
        for mc in range(nmc):
            nc.sync.dma_start(wo2[mc][:], wo2_in[mc * 128:(mc + 1) * 128, :])
            nc.sync.dma_start(wd2[mc][:], wd2_in[mc * 128:(mc + 1) * 128, :])
        boy = cpool.tile([24, 1], F32, tag="boy")
        nc.sync.dma_start(boy[:], bo2_in[0:24, :])
        box32 = cpool.tile([56, 1], F32, tag="box32")
        nc.sync.dma_start(box32[32:56], bo2_in[24:48, :])
        ident = cpool.tile([128, 128], BF16, tag="ident")
        nc.sync.dma_start(ident[:], idn_in[:])


        for b in range(bpc):
            for lc in range(nlc):
                l0 = lc * lchunk
                # ---- load: one DMA per chunk (halo'd windows) ----
                W = 16 + M + 16
                xbw = lpool.tile([128, nct * W], BF16, tag="xbw",
                                 bufs=g("xb", 3))
                # one DMA for the whole chunk: overlapping 416-wide windows
                src = x_nat[b, 1 + l0:1 + l0 + 128, :]
                src = src[:, None, :].broadcast_to((128, nct, M))
                # [p, i, w]: p step=M rows, i step=128*M rows, w step 1,
                # offset shifted -16 for the left halo
                src.ap = mybir.VecI64Pair(
                    [[M, 128], [128 * M, nct], [1, W]])
                src.offset = src.offset - 16
                dst = xbw[:].rearrange("p (i w) -> p i w", i=nct)
                nc.sync.dma_start(dst, src)
                xbs = [xbw[:, i * W:i * W + W] for i in range(nct)]

                # ---- T1: transpose raw x to [m, l] for the offset conv ----
                xt = []
                for mc in range(nmc):
                    tr = trpool.tile([128, lchunk], BF16, tag="tr")
                    for i in range(nct):
                        nc.tensor.transpose(
                            tr[:, i * 128:(i + 1) * 128],
                            xbs[i][:, 16 + mc * 128:16 + (mc + 1) * 128],
                            ident[:])
                    xts = wpool.tile([128, lchunk], BF16, tag="xt", bufs=g("xt", 4))
                    nc.scalar.copy(xts[:], tr[:])
                    xt.append(xts)
                if dbg and b == 0 and lc == 0:
                    nc.sync.dma_start(dbg_xt[:], xt[0][:])

                # ---- stage A: offset conv ----
                offps = offpool.tile([64, lchunk], F32, tag="offps")
                for mc in range(nmc):
                    nc.tensor.matmul(offps[:], wo2[mc][:], xt[mc][:],
                                     start=(mc == 0), stop=(mc == nmc - 1))
                dyt = wpool.tile([24, lchunk], BF16, tag="dyt")
                nc.scalar.activation(dyt[:], offps[0:24],
                                     mybir.ActivationFunctionType.Identity,
                                     bias=boy[:], scale=1.0)
                if dbg and b == 0 and lc == 0:
                    nc.sync.dma_start(dbg_dy[:], dyt[:])

                # ---- u pipeline, all on partitions 32:56 ----
                dy32 = wpool.tile([56, lchunk], BF16, tag="dy32")
                nc.sync.dma_start(dy32[32:56], dyt[:])
                dy = dy32[32:56]
                s1 = wpool.tile([56, lchunk], BF16, tag="s1", name="s1")[32:56]
                nc.vector.tensor_scalar(s1, dy, -1.0, 1.0, TT.mult, TT.add)
                s2 = wpool.tile([56, lchunk], BF16, tag="s2", name="s2")[32:56]
                nc.vector.tensor_scalar_add(s2, dy, 1.0)
                wy = wpool.tile([56, lchunk], BF16, tag="wy", name="wy")[32:56]
                nc.vector.tensor_tensor(out=wy, in0=s1, in1=s2, op=TT.min)
                nc.vector.tensor_scalar_max(wy, wy, 0.0)
                dxt = wpool.tile([56, lchunk], BF16, tag="dxt", name="dxt")[32:56]
                nc.vector.tensor_scalar(dxt, offps[32:56], 1.0, box32[32:56],
                                        TT.mult, TT.add)
                rp = wpool.tile([56, lchunk], BF16, tag="rp", name="rp")[32:56]
                nc.vector.tensor_scalar_max(rp, dxt, 0.0)
                rm = wpool.tile([56, lchunk], BF16, tag="rm", name="rm")[32:56]
                nc.vector.tensor_scalar(rm, dxt, -1.0, 0.0, TT.mult, TT.max)
                u_m = wpool.tile([56, lchunk], BF16, tag="u_m", name="u_m")[32:56]
                u_0 = wpool.tile([56, lchunk], BF16, tag="u_0", name="u_0")[32:56]
                u_p = wpool.tile([56, lchunk], BF16, tag="u_p", name="u_p")[32:56]
                nc.vector.tensor_tensor(out=u_p, in0=wy, in1=rp, op=TT.mult)
                nc.vector.tensor_tensor(out=u_m, in0=wy, in1=rm, op=TT.mult)
                t0 = wpool.tile([56, lchunk], BF16, tag="t0", name="t0")[32:56]
                nc.vector.tensor_tensor(out=t0, in0=wy, in1=u_p, op=TT.subtract)
                nc.vector.tensor_tensor(out=u_0, in0=t0, in1=u_m, op=TT.subtract)
                u_tiles = [u_m, u_0, u_p]

                # ---- transpose u3 to natural [l, (j k)] ----
                upsum = upool.tile([128, 72 * nct], BF16, tag="upsum")
                for i in range(nct):
                    for jj in range(3):
                        nc.tensor.transpose(
                            upsum[:, i * 72 + jj * 24:i * 72 + jj * 24 + 24],
                            u_tiles[jj][:, i * 128:(i + 1) * 128],
                            ident[32:56, 32:56])
                u3n = wpool.tile([128, 144 * nct], BF16, tag="u3n")
                usrc = upsum[:].rearrange("p (i r) -> p i r", i=nct)
                usrc = usrc[:, :, :, None].broadcast_to((128, nct, 72, 2))
                nc.scalar.copy(
                    u3n[:].rearrange("p (i r t) -> p i r t", i=nct, r=72),
                    usrc)
                if dbg and b == 0 and lc == 0:
                    nc.sync.dma_start(dbg_u3n[:], u3n[:])

                # ---- weighting (natural domain) + T2 per l-tile ----
                vt = []
                for mc in range(nmc):
                    t_vt = t2pool.tile([128, lchunk], BF16, tag="tr2", name=f"vt_{mc}")
                    vt.append(t_vt)
                for i in range(nct):
                    xb = xbs[i]
                    ub = u3n[:, i * 144:(i + 1) * 144]
                    pbuf = vpool.tile([128, 3 * M], BF16, tag="pbuf")
                    for jj in range(3):
                        # U pairs: u3 rows 32*jj.. -> cols 64*jj..64*jj+48,
                        # each value duplicated (2k, 2k+1)
                        uv = ub[:, 48 * jj:48 * jj + 48].rearrange(
                            "p (k t) -> p k t", k=K)
                        uv = uv[:, :, None, :].broadcast_to((128, K, 8, 2))
                        xv = xb[:, 16 * jj:16 * jj + M].rearrange(
                            "p (k c8 c2) -> p k c8 c2", k=K, c8=8)
                        pv = pbuf[:, jj * M:(jj + 1) * M].rearrange(
                            "p (k c8 c2) -> p k c8 c2", k=K, c8=8)
                        nc.vector.tensor_tensor(out=pv, in0=xv, in1=uv,
                                                op=TT.mult)
                    v2w = vpool.tile([128, M], BF16, tag="v2w")
                    add_eng = nc.gpsimd if g("gadd", 0) else nc.vector
                    add_eng.tensor_tensor(out=v2w[:], in0=pbuf[:, 0:M],
                                          in1=pbuf[:, M:2 * M], op=TT.add)
                    add_eng.tensor_tensor(out=v2w[:], in0=v2w[:],
                                          in1=pbuf[:, 2 * M:3 * M], op=TT.add)
                    if dbg and b == 0 and lc == 0 and i == 0:
                        nc.sync.dma_start(dbg_v2w[:], v2w[:])
                    for mc in range(nmc):
                        nc.tensor.transpose(
                            vt[mc][:, i * 128:(i + 1) * 128],
                            v2w[:, mc * 128:(mc + 1) * 128],
                            ident[:])
                vts = []
                for mc in range(nmc):
                    v = vpool.tile([128, lchunk], BF16, tag="vts", bufs=g("vts", 4))
                    nc.scalar.copy(v[:], vt[mc][:])
                    vts.append(v)
                if dbg and b == 0 and lc == 0:
                    nc.sync.dma_start(dbg_vts[:], vts[0][:])
                # ---- stage C ----
                osb4 = vpool.tile([128, nct * d], BF16, tag="osb4",
                                  bufs=g("osb", 2))
                for i in range(nct):
                    outps = opool.tile([128, d], F32, tag="outps")
                    for mc in range(nmc):
                        nc.tensor.matmul(outps[:],
                                         vts[mc][:, i * 128:(i + 1) * 128],
                                         wd2[mc][:],
                                         start=(mc == 0),
                                         stop=(mc == nmc - 1))
                    nc.scalar.copy(osb4[:, i * d:(i + 1) * d], outps[:])
                odst = out_dram[b, l0:l0 + lchunk, :].rearrange(
                    "(i p) d -> p i d", p=128)
                nc.sync.dma_start(odst, osb4[:].rearrange(
                    "p (i d) -> p i d", i=nct))

    nc.compile()
    return nc


def prep_weights(w_off, b_off, w_def):
    """Host-side weight rearrangement. wo2[k*C+c, o'] with o' 0..23 = dy_k
    (w_off channel 2k), o' 24..47 = dx_k (channel 2k+1)."""
    d = w_def.shape[0]
    wo2 = np.zeros((M, 64), np.float32)
    wd2 = np.zeros((M, d), np.float32)
    bo2 = np.zeros((48, 1), np.float32)
    for k in range(K):
        for c in range(C):
            m = k * C + c
            wo2[m, 0:24] = w_off[0::2, c, k]
            wo2[m, 32:56] = w_off[1::2, c, k]
            wd2[m, :] = w_def[:, c, k]
    bo2[0:24, 0] = b_off[0::2]
    bo2[24:48, 0] = b_off[1::2]
    return (wo2.astype(ml_dtypes.bfloat16), bo2,
            wd2.astype(ml_dtypes.bfloat16))


def make_identity():
    return np.eye(128, dtype=ml_dtypes.bfloat16)


def make_identity2():
    i2 = np.zeros((96, 192), ml_dtypes.bfloat16)
    for r in range(96):
        i2[r, 2 * r] = 1
        i2[r, 2 * r + 1] = 1
    return i2


_NC_CACHE = {}


def pad_x(x_shard):
    bpc = x_shard.shape[0]
    lout = x_shard.shape[1] // K
    xp = np.zeros((bpc, lout + 2, M), ml_dtypes.bfloat16)
    xp[:, 1:-1, :] = x_shard.reshape(bpc, lout, M).astype(ml_dtypes.bfloat16)
    return xp


def kernel(x, w_off, b_off, w_def, b_def, trace=False):
    x = np.ascontiguousarray(np.asarray(x, np.float32))
    wo2, bo2, wd2 = prep_weights(np.asarray(w_off, np.float32),
                                 np.asarray(b_off, np.float32),
                                 np.asarray(w_def, np.float32))
    idn = make_identity()
    if "nc" not in _NC_CACHE:
        _NC_CACHE["nc"] = build_kernel()
    nc = _NC_CACHE["nc"]
    in_maps = []
    for r in range(NCORES):
        in_maps.append({
            "x": pad_x(x[r * BPC:(r + 1) * BPC]),
            "wo2": wo2, "bo2": bo2, "wd2": wd2, "idn": idn,
        })
    try:
        res = run_bass_kernel_spmd(nc, in_maps, core_ids=list(range(NCORES)),
                                   trace=trace)
    except (ImportError, ModuleNotFoundError):
        res = run_bass_kernel_spmd(nc, in_maps, core_ids=list(range(NCORES)))
    out = np.concatenate([np.asarray(res.results[r]["out"], np.float32)
                          for r in range(NCORES)], axis=0)
    out = out + np.asarray(b_def, np.float32)[None, None, :]
    if trace:
        return out.astype(np.float32), res
    return out.astype(np.float32)

